# revision 35
# baseline (speedup 1.0000x reference)
"""Trainium2 Bass kernel for nn_BioV_19748259627109 (v2).

Pipeline per core (data-parallel over batch B=8, one sample per core):
  S1  spatial 3x3 conv (1->3ch) as PE band-matmuls over H, f32r; silu -> bf16
  EX  per-(c,q) DRAM-bounced layout exchange [h,(t,w)] -> [(q,t),(c,hq,w)],
      software-pipelined per channel so S2(c) overlaps S1(c+1)
  S2  temporal depthwise conv (7 taps) as block-diag PE matmuls, bf16
  S3  g = silu(silu(conv)) on ACT with fused per-partition stat sidebands
  ST  partition-sum of stats via a single PE ones-matmul; 6-float payload
  AR  AllReduce (batch-norm terms) triggered right after the last silu;
      all kv machinery (kv_s PE, kv_t DVE dots, softmax maxes, act-table
      preload, PSUM evacuations) hides inside the ~45us collective window
  OUT rank-1 outer product At (x) As in bf16 on DVE/GPSIMD; output written
      as [h,(c,t,w)] bf16 with per-partition-contiguous descriptors; host
      upcasts and transposes to [c,t,h,w] f32.

out[c,t,s] = At[c,t]*As[c,s] exactly (GainControl factors rank-1), and
SwitchNorm is per-(b,c) affine commuting with the kv contractions, so the
normalized tensor is never materialized.
"""
import sys
if '/opt/trn_rl_repo' not in sys.path:
    sys.path.insert(0, '/opt/trn_rl_repo')

import numpy as np
from concourse import bass, bacc, tile, mybir

F32 = mybir.dt.float32
F32R = mybir.dt.float32r
BF16 = mybir.dt.bfloat16
BF16_NP = mybir.dt.np(BF16)
ALU = mybir.AluOpType
AFT = mybir.ActivationFunctionType
AXT = mybir.AxisListType

N_CORES = 8
B, T, H, W = 8, 32, 128, 128
C = 3
NTOT = float(T * H * W)
EPS = 1e-5


def _host_constants(inputs):
    w_s = np.asarray(inputs['w_spatial'], np.float32)     # (3,1,3,3)
    b_s = np.asarray(inputs['b_spatial'], np.float32)
    w_t = np.asarray(inputs['w_temporal'], np.float32)    # (3,1,7,1)
    b_t = np.asarray(inputs['b_temporal'], np.float32)
    sn_w = np.asarray(inputs['sn_weight'], np.float32).reshape(3)
    sn_b = np.asarray(inputs['sn_bias'], np.float32).reshape(3)
    mwr = np.asarray(inputs['mean_weight'], np.float32)
    vwr = np.asarray(inputs['var_weight'], np.float32)
    mw = np.exp(mwr - mwr.max()); mw = mw / mw.sum()
    vw = np.exp(vwr - vwr.max()); vw = vw / vw.sum()
    wkvs = np.asarray(inputs['w_kv_s'], np.float32)       # (2,32)
    wkvt = np.asarray(inputs['w_kv_t'], np.float32)       # (2,16384)

    # bandW[h_in, c, dx, h_out] = w_s[c,0,h_in-h_out+1,dx]
    hi = np.arange(128)[:, None]
    ho = np.arange(128)[None, :]
    dy = hi - ho + 1
    bandw = np.zeros((128, 3, 3, 128), np.float32)
    for c in range(3):
        for dx in range(3):
            m = np.where((dy >= 0) & (dy <= 2), w_s[c, 0, np.clip(dy, 0, 2), dx], 0.0)
            bandw[:, c, dx, :] = m.astype(np.float32)

    # bandT[(q,t_in), c, (q,t_out)] block-diagonal over quarters, bf16
    ti = np.arange(32)[:, None]
    to = np.arange(32)[None, :]
    kk = ti - to + 3
    bandt32 = np.zeros((32, 3, 32), np.float32)
    for c in range(3):
        bandt32[:, c, :] = np.where((kk >= 0) & (kk <= 6), w_t[c, 0, np.clip(kk, 0, 6), 0], 0.0)
    bandt = np.zeros((128, 3, 128), np.float32)
    for q in range(4):
        bandt[32 * q:32 * q + 32, :, 32 * q:32 * q + 32] = bandt32
    bandt = bandt.astype(BF16_NP)

    # kv_s lhsT [(q,t)=128, (o,q0)=8] -- o-major so evac rows are contiguous
    kvs_lhst = np.zeros((128, 8), np.float32)
    for q in range(4):
        for t in range(32):
            for o in range(2):
                kvs_lhst[q * 32 + t, o * 4 + q] = wkvs[o, t]
    kvs_lhst = kvs_lhst.astype(BF16_NP)

    # qsum[p=(q,t), t0] = 1 iff t == t0: partition-sum over quarters keeping t
    qsum = np.zeros((128, 32), np.float32)
    qsum[np.arange(128), np.arange(128) % 32] = 1.0

    wkvt4 = wkvt.reshape(2, 4, 32, 128).transpose(1, 0, 2, 3).astype(BF16_NP)[None]  # (1,q,o,hq,w)

    ws_sum = wkvs.sum(axis=1)   # (2,)
    wt_sum = wkvt.sum(axis=1)   # (2,)
    # crow layout: [0:3] sn_w, [3:6] sn_b, [6:9] Ws1 rep, [9:12] Wt1 rep
    crow = np.zeros((1, 32), np.float32)
    crow[0, 0:3] = sn_w
    crow[0, 3:6] = sn_b
    crow[0, 6:9] = ws_sum[1]
    crow[0, 9:12] = wt_sum[1]
    scal = dict(
        b_s=[float(v) for v in b_s], b_t=[float(v) for v in b_t],
        mw=[float(v) for v in mw], vw=[float(v) for v in vw],
    )
    return dict(bandw=bandw, bandt=bandt, kvs_lhst=kvs_lhst, qsum=qsum,
                wkvt4=wkvt4, crow=crow, scal=scal)


def build_program(scal, no_cc=False):
    nc = bacc.Bacc("TRN2", target_bir_lowering=False, debug=False,
                   num_devices=N_CORES)

    xin = nc.dram_tensor("xin", [128, 32, 130], F32R, kind="ExternalInput")
    bandw_d = nc.dram_tensor("bandw", [128, 3, 3, 128], F32R, kind="ExternalInput")
    bandt_d = nc.dram_tensor("bandt", [128, 3, 128], BF16, kind="ExternalInput")
    kvsl_d = nc.dram_tensor("kvs_lhst", [128, 8], BF16, kind="ExternalInput")
    qsum_d = nc.dram_tensor("qsum", [128, 32], F32, kind="ExternalInput")
    wkvt_d = nc.dram_tensor("wkvt4", [1, 4, 2, 32, 128], BF16, kind="ExternalInput")
    crow_d = nc.dram_tensor("crow", [1, 32], F32, kind="ExternalInput")
    # output stays in [h, c, t, w] so every partition writes one contiguous run;
    # the host transposes back to [c, t, h, w].
    out_d = nc.dram_tensor("out", [128, 3, 32, 128], BF16, kind="ExternalOutput")

    b_s, b_t = scal['b_s'], scal['b_t']
    mw, vw = scal['mw'], scal['vw']
    invN = 1.0 / NTOT
    aN = NTOT / (NTOT - 1.0)          # unbiased-variance factor
    bN = 1.0 / (NTOT - 1.0)

    with tile.TileContext(nc) as tc:
        with (
            tc.tile_pool(name="const", bufs=1) as cpool,
            tc.tile_pool(name="big", bufs=1) as bigp,
            tc.tile_pool(name="yc", bufs=2) as ycp,
            tc.tile_pool(name="work", bufs=2) as wpool,
            tc.tile_pool(name="psum", bufs=2, space="PSUM") as pp,
            tc.tile_pool(name="dram", bufs=1, space="DRAM") as dram,
        ):
            # ---- constants + input loads ----
            x_sb = bigp.tile([128, 32, 130], F32R, tag="xbig")
            nc.sync.dma_start(x_sb[:, 0:16, :], xin[:, 0:16, :])
            nc.sync.dma_start(x_sb[:, 16:32, :], xin[:, 16:32, :])
            bandw_sb = cpool.tile([128, 3, 3, 128], F32R)
            nc.sync.dma_start(bandw_sb[:], bandw_d[:])
            bandt_sb = cpool.tile([128, 3, 128], BF16)
            nc.sync.dma_start(bandt_sb[:], bandt_d[:])
            kvsl_sb = cpool.tile([128, 8], BF16)
            nc.sync.dma_start(kvsl_sb[:], kvsl_d[:])
            qsum_sb = cpool.tile([128, 32], F32)
            nc.sync.dma_start(qsum_sb[:], qsum_d[:])
            crow_sb = cpool.tile([1, 32], F32)
            nc.sync.dma_start(crow_sb[:], crow_d[:])
            bvals = cpool.tile([128, 8], F32)
            for c in range(3):
                nc.vector.memset(bvals[:, c:c + 1], b_s[c])
                nc.vector.memset(bvals[:, 3 + c:4 + c], b_t[c])
            nc.vector.memset(bvals[:, 6:7], EPS)
            ones1 = cpool.tile([128, 1], F32)
            nc.vector.memset(ones1[:], 1.0)
            ones_row = cpool.tile([1, 128], F32)
            nc.vector.memset(ones_row[:], 1.0)
            magic = cpool.tile([1, 8], mybir.dt.int32)
            nc.vector.memset(magic[:], 0x5f3759df)
            c15 = cpool.tile([1, 8], F32)
            nc.vector.memset(c15[:], 1.5)
            dum = cpool.tile([1, 2], F32)
            nc.vector.memset(dum[:], 0.0)
            # preload silu act table before the first real silu
            nc.scalar.activation(dum[:, 1:2], dum[:, 0:1], AFT.Silu)

            def rsqrt_row(out, in_, n, iscr, fscr):
                """out = in_^-1/2 on DVE (Quake seed + 2 Newton steps); avoids
                the ACT Ln/Exp table swaps. iscr: int32 scratch [1, >=n];
                fscr: f32 scratch [1, >=2n]."""
                I32 = mybir.dt.int32
                nc.vector.tensor_scalar(iscr[:, 0:n], in_.bitcast(I32),
                                        1, None, ALU.logical_shift_right)
                nc.vector.tensor_sub(out.bitcast(I32), magic[:, 0:n],
                                     iscr[:, 0:n])
                y = out
                for _ in range(2):
                    nc.vector.tensor_mul(fscr[:, 0:n], y, y)
                    nc.vector.tensor_mul(fscr[:, n:2 * n], fscr[:, 0:n], in_)
                    nc.vector.scalar_tensor_tensor(
                        fscr[:, n:2 * n], fscr[:, n:2 * n], -0.5,
                        c15[:, 0:n], ALU.mult, ALU.add)
                    nc.vector.tensor_mul(y, y, fscr[:, n:2 * n])

            ydram = dram.tile([3, 4, 32, 32, 128], BF16)     # [c, q, t, hq, w]
            yB = bigp.tile([128, 3, 32, 128], BF16)          # [(q,t), c, hq, w]
            gB = bigp.tile([128, 3, 32, 128], BF16)          # [(q,t), c, hq, w]
            accs = cpool.tile([128, 12], F32)
            kvs_tmp = bigp.tile([8, 4160], F32, tag="kvstmp")
            kvsA = cpool.tile([128, 3, 2, 128], F32)
            kvt_acc = cpool.tile([128, 8], F32)
            sc = cpool.tile([1, 32], F32)
            sc2 = cpool.tile([1, 32], F32)
            arow = cpool.tile([1, 32], F32)
            nc.vector.memset(sc[:, 22:24], 0.0)              # AR payload padding

            # wkvt broadcast (2MB SBUF) -- only needed by kv_t dots, which run
            # inside the collective window; issue after the exchange DMAs.
            wkvt_sb = bigp.tile([128, 2, 32, 128], BF16, tag="late")

            def s1(c):
                # exchange writes happen per (c, q, half) right after each
                # half's silu: the slow (256B-descriptor) transposed side rides
                # the writes, which overlap the next S1 chunk's compute.
                ycb = ycp.tile([128, 32, 128], BF16, tag="ycb")
                for half in range(2):
                    t0 = 16 * half
                    ps = pp.tile([128, 2048], F32, tag="mm")
                    for j in range(4):
                        for dx in range(3):
                            nc.tensor.matmul(
                                ps[:, 512 * j:512 * (j + 1)],
                                lhsT=bandw_sb[:, c, dx, :],
                                rhs=x_sb[:, t0 + 4 * j:t0 + 4 * j + 4, dx:dx + 128],
                                start=(dx == 0), stop=(dx == 2),
                            )
                    nc.scalar.activation(
                        ycb[:, t0:t0 + 16, :].rearrange("p a b -> p (a b)"),
                        ps[:], AFT.Silu, bias=bvals[:, c:c + 1])
                # transposed (slow-descriptor) side rides the writes, on SP
                for q in range(4):
                    nc.sync.dma_start(ydram[c, q].transpose([1, 0, 2]),
                                      ycb[32 * q:32 * q + 32, :, :])
                # fast contiguous reads, issued from the idle GPSIMD queue to
                # keep the SP sequencer's per-DMA DGE overhead off the
                # critical path
                for q in range(4):
                    nc.gpsimd.dma_start(yB[32 * q:32 * q + 32, c, :, :],
                                        ydram[c, q])

            def s2(c):
                for half in range(2):
                    hq0 = 16 * half
                    ps = pp.tile([128, 2048], F32, tag="mm")
                    for j in range(4):
                        nc.tensor.matmul(
                            ps[:, 512 * j:512 * (j + 1)],
                            lhsT=bandt_sb[:, c, :],
                            rhs=yB[:, c, hq0 + 4 * j:hq0 + 4 * j + 4, :],
                            start=True, stop=True,
                        )
                    zscr = wpool.tile([128, 2048], F32, tag="zscr")
                    nc.scalar.activation(zscr[:], ps[:], AFT.Silu,
                                         bias=bvals[:, 3 + c:4 + c])
                    nc.scalar.activation(
                        gB[:, c, hq0:hq0 + 16, :].rearrange("p a b -> p (a b)"),
                        zscr[:], AFT.Silu,
                        accum_out=accs[:, 2 * c + half:2 * c + half + 1])

            def s2_square(c):
                for half in range(2):
                    hq0 = 16 * half
                    sq = wpool.tile([128, 2048], BF16, tag="sq")
                    gsl = gB[:, c, hq0:hq0 + 16, :].rearrange("p a b -> p (a b)")
                    nc.vector.scalar_tensor_tensor(
                        sq[:], gsl, 1.0, gsl, ALU.mult, ALU.mult,
                        accum_out=accs[:, 6 + 2 * c + half:7 + 2 * c + half])

            # ---- software-pipelined: all S1 (exchange rides each c), then S2 ----
            s1(0)
            s1(1)
            s1(2)
            s2(0)
            s2_square(0)
            s2(1)
            s2_square(1)
            s2(2)
            s2_square(2)

            # ---- stats: PE partition-sum + short DVE chain -> AR ----
            hp = tc.high_priority()
            hp.__enter__()
            ps_st = pp.tile([1, 12], F32, tag="mm")
            nc.tensor.matmul(ps_st[:], lhsT=ones1[:], rhs=accs[:, 0:12],
                             start=True, stop=True)
            # sc slots: [0:12] raw sums, [12:15] sum_g, [15:18] sum_g2,
            # [16:19]? no -- payload lives at [16:24]: mean_in [16:19],
            # temp [19:22], pad [22:24]; scratch [24:32] = AR result.
            nc.vector.tensor_copy(sc[:, 0:12], ps_st[:])
            nc.vector.tensor_add(sc2[:, 6:12], sc[:, 0:12:2], sc[:, 1:12:2])
            nc.vector.tensor_scalar_mul(sc[:, 16:19], sc2[:, 6:9], invN)    # mean_in
            nc.vector.tensor_mul(sc2[:, 0:3], sc[:, 16:19], sc[:, 16:19])  # msq
            nc.vector.tensor_scalar_mul(sc2[:, 3:6], sc2[:, 0:3], -bN)
            nc.vector.scalar_tensor_tensor(sc[:, 19:22], sc2[:, 9:12], aN * invN,
                                           sc2[:, 3:6], ALU.mult, ALU.add)  # temp
            # local (AR-independent) SwitchNorm terms, computed pre-AR:
            nc.vector.tensor_reduce(sc2[:, 15:16], sc[:, 16:19], AXT.X, ALU.add)
            nc.vector.tensor_scalar_mul(sc2[:, 15:16], sc2[:, 15:16], 1.0 / 3)  # mean_ln
            nc.vector.tensor_reduce(sc2[:, 16:17], sc[:, 19:22], AXT.X, ALU.add)
            nc.vector.tensor_scalar_mul(sc2[:, 16:17], sc2[:, 16:17], 1.0 / 3)  # Etemp_l
            nc.vector.tensor_mul(sc2[:, 17:18], sc2[:, 15:16], sc2[:, 15:16])
            nc.vector.tensor_sub(sc2[:, 17:18], sc2[:, 16:17], sc2[:, 17:18])  # var_ln
            # pre_mean [18:21] = mw0*mean_in + mw1*mean_ln
            nc.vector.tensor_scalar_mul(sc2[:, 26:27], sc2[:, 15:16], mw[1])
            nc.vector.tensor_scalar(sc2[:, 18:21], sc[:, 16:19], mw[0],
                                    sc2[:, 26:27], ALU.mult, ALU.add)
            # var_in = temp - msq
            nc.vector.tensor_sub(sc2[:, 21:24], sc[:, 19:22], sc2[:, 0:3])
            # pre_var+eps [27:30] = vw0*var_in + (vw1*var_ln + eps)
            nc.vector.tensor_scalar(sc2[:, 30:31], sc2[:, 17:18], vw[1],
                                    EPS, ALU.mult, ALU.add)
            nc.vector.tensor_scalar(sc2[:, 27:30], sc2[:, 21:24], vw[0],
                                    sc2[:, 30:31], ALU.mult, ALU.add)

            cc_in = dram.tile([1, 8], F32)
            cc_out = dram.tile([1, 8], F32)
            nc.scalar.dma_start(cc_in[:], sc[:, 16:24])
            if no_cc:
                nc.sync.dma_start(cc_out[:], cc_in[:])
            else:
                nc.gpsimd.collective_compute(
                    "AllReduce", ALU.add,
                    replica_groups=[list(range(N_CORES))],
                    ins=[cc_in.opt()], outs=[cc_out.opt()])
            # Gate the wkvt broadcast (2MB) and the kv_s evacuations on the
            # stats chain. The scheduler hoists any dependency-free work to
            # t=0, which starved the x-load DMA and queued the evacuations
            # ahead of the collective payload; writing stats bytes into the
            # gate regions forces everything downstream behind the trigger.
            nc.vector.tensor_copy(kvs_tmp[0:1, 0:1], sc[0:1, 19:20])
            nc.vector.tensor_copy(kvs_tmp[0:1, 2048:2049], sc[0:1, 19:20])
            wkstage = dram.tile([4, 2, 32, 128], BF16)
            # gate on the stats row: the 2MB broadcast stays out of phase A's
            # DMA-saturated window and lands inside the collective instead
            nc.scalar.dma_start(wkstage[0, 0, 0, 0:16],
                                sc[:, 16:24].bitcast(BF16)[:, 0:16])
            nc.gpsimd.dma_start(wkstage[:], wkvt_d[0])
            hp.__exit__(None, None, None)

            # preload the exp act table during the collective (input sourced
            # from the stats row so it cannot be hoisted between the silus)
            nc.scalar.activation(dum[:, 1:2], sc[0:1, 19:20], AFT.Exp)

            for q in range(4):
                nc.gpsimd.dma_start(
                    wkvt_sb[32 * q:32 * q + 32, :, :, :],
                    wkstage[q].unsqueeze(0).broadcast_to([32, 2, 32, 128]),
                )

            # ---- kv_s contraction (PE) + DVE evac + flat-DMA scatter ----
            for c in range(3):
                for half in range(2):
                    hq0 = 16 * half
                    ps = pp.tile([8, 2048], F32, tag="mm")
                    for j in range(4):
                        nc.tensor.matmul(
                            ps[:, 512 * j:512 * (j + 1)],
                            lhsT=kvsl_sb[:],
                            rhs=gB[:, c, hq0 + 4 * j:hq0 + 4 * j + 4, :],
                            start=True, stop=True)
                    nc.scalar.copy(
                        kvs_tmp[:, 2048 * half:2048 * (half + 1)], ps[:])
                for o in range(2):
                    nc.sync.dma_start(kvsA[:, c, o, :],
                                      kvs_tmp[4 * o:4 * o + 4, 0:4096])

            # ---- kv_t row dots (DVE, in the AR window) ----
            for o in range(2):
                for c in range(3):
                    sq2 = wpool.tile([128, 4096], BF16, tag="kvtscr")
                    nc.vector.scalar_tensor_tensor(
                        sq2[:], gB[:, c].rearrange("p hq w -> p (hq w)"), 1.0,
                        wkvt_sb[:, o].rearrange("p hq w -> p (hq w)"),
                        ALU.mult, ALU.mult,
                        accum_out=kvt_acc[:, 3 * o + c:3 * o + c + 1])
            ps_kvt = pp.tile([6, 32], F32, tag="mm")
            nc.tensor.matmul(ps_kvt[:], lhsT=kvt_acc[:, 0:6], rhs=qsum_sb[:],
                             start=True, stop=True)
            kvt6 = cpool.tile([6, 32], F32)
            nc.vector.tensor_copy(kvt6[:], ps_kvt[:])
            ktrow = cpool.tile([1, 192], F32)   # (o,c,t)
            nc.sync.dma_start(ktrow[:, 0:192], kvt6[:])

            # (no softmax max-subtraction: |alpha*kv| <= ~12, exp is safe in f32)

            # AR result readback: emitted here so the SP queue's blocking wait
            # on the collective comes AFTER all AR-window DMAs (kvsA, ktrow).
            nc.sync.dma_start(sc[:, 24:32], cc_out[:])

            # ---- post-AR scalar math: batch-norm terms + mix-in ----
            # dataflow fence: the scheduler's collective cost model is
            # optimistic, which otherwise commits the post-AR DVE ops ahead
            # of the kv_t dots in the in-order queue; reading kvt_acc here
            # forces kv_t (and everything in the AR window) to come first
            nc.vector.tensor_copy(sc2[0:1, 6:12], kvt_acc[0:1, 0:6])
            nc.vector.tensor_scalar_mul(sc2[:, 6:9], sc[:, 24:27], 1.0 / B)   # mean_bn
            nc.vector.tensor_scalar_mul(sc2[:, 9:12], sc[:, 27:30], 1.0 / B)  # Etemp_b
            nc.vector.tensor_mul(sc2[:, 12:15], sc2[:, 6:9], sc2[:, 6:9])
            nc.vector.tensor_sub(sc2[:, 12:15], sc2[:, 9:12], sc2[:, 12:15])  # var_bn
            # mean [18:21] = pre_mean + mw2*mean_bn
            nc.vector.scalar_tensor_tensor(sc2[:, 18:21], sc2[:, 6:9], mw[2],
                                           sc2[:, 18:21], ALU.mult, ALU.add)
            # var+eps [27:30] = pre_var+eps + vw2*var_bn
            nc.vector.scalar_tensor_tensor(sc2[:, 27:30], sc2[:, 12:15], vw[2],
                                           sc2[:, 27:30], ALU.mult, ALU.add)
            # rstd = (var+eps)^-1/2 on DVE -- no ACT table swap
            iscr = cpool.tile([1, 8], mybir.dt.int32)
            fscr = cpool.tile([1, 16], F32)
            rsqrt_row(sc2[:, 24:27], sc2[:, 27:30], 3, iscr, fscr)
            # arow: [0:3] alpha, [3:6] 0.5*alpha, [6:9] beta*Ws1, [9:12] beta*Wt1
            nc.vector.tensor_mul(arow[:, 0:3], sc2[:, 24:27], crow_sb[:, 0:3])
            nc.vector.tensor_scalar_mul(arow[:, 3:6], arow[:, 0:3], 0.5)
            nc.vector.tensor_mul(arow[:, 12:15], sc2[:, 18:21], arow[:, 0:3])
            nc.vector.tensor_sub(arow[:, 12:15], crow_sb[:, 3:6], arow[:, 12:15])  # beta
            nc.vector.tensor_mul(arow[:, 6:9], arow[:, 12:15], crow_sb[:, 6:9])
            nc.vector.tensor_mul(arow[:, 9:12], arow[:, 12:15], crow_sb[:, 9:12])
            ab_rep = cpool.tile([128, 12], F32)
            ps_ab = pp.tile([128, 12], F32, tag="mm")
            nc.tensor.matmul(ps_ab[:], lhsT=ones_row[:], rhs=arow[:, 0:12],
                             start=True, stop=True)
            nc.vector.tensor_copy(ab_rep[:], ps_ab[:])

            # ---- As: exp(alpha*(v-m)/2) * (alpha*vs + beta*Ws1) / sqrt(Z) ----
            ehraw = cpool.tile([128, 3, 128], F32)
            for c in range(3):
                nc.scalar.activation(ehraw[:, c], kvsA[:, c, 0, :], AFT.Exp,
                                     scale=ab_rep[:, 3 + c:4 + c])
            vsaff = cpool.tile([128, 3, 128], F32)
            nc.vector.tensor_mul(
                vsaff[:], kvsA[:, :, 1, :],
                ab_rep[:, 0:3].unsqueeze(2).broadcast_to([128, 3, 128]))
            nc.vector.tensor_add(
                vsaff[:], vsaff[:],
                ab_rep[:, 6:9].unsqueeze(2).broadcast_to([128, 3, 128]))
            zacc = cpool.tile([128, 4], F32)
            zscr2 = cpool.tile([128, 128], F32)
            for c in range(3):
                nc.vector.scalar_tensor_tensor(
                    zscr2[:], ehraw[:, c], 1.0, ehraw[:, c], ALU.mult, ALU.mult,
                    accum_out=zacc[:, c:c + 1])
            ps_z = pp.tile([1, 3], F32, tag="mm")
            nc.tensor.matmul(ps_z[:], lhsT=ones1[:], rhs=zacc[:, 0:3],
                             start=True, stop=True)

            # ---- At: same math on the 96-value t-row ----
            ehalft = cpool.tile([1, 96], F32)
            for c in range(3):
                nc.scalar.activation(ehalft[:, 32 * c:32 * c + 32],
                                     ktrow[:, 32 * c:32 * c + 32], AFT.Exp,
                                     scale=arow[:, 3 + c:4 + c])
            ztacc = cpool.tile([1, 4], F32)
            ztscr = cpool.tile([1, 32], F32)
            for c in range(3):
                nc.vector.scalar_tensor_tensor(
                    ztscr[:], ehalft[:, 32 * c:32 * c + 32], 1.0,
                    ehalft[:, 32 * c:32 * c + 32], ALU.mult, ALU.mult,
                    accum_out=ztacc[:, c:c + 1])
            # batched Z^-1/2 (As, cols 0:3) and Zt^-1/2 (At, cols 3:6) on DVE
            zzrow = cpool.tile([1, 8], F32)
            nc.vector.tensor_copy(zzrow[:, 0:3], ps_z[:])
            nc.vector.tensor_copy(zzrow[:, 3:6], ztacc[:, 0:3])
            zzinv = cpool.tile([1, 8], F32)
            rsqrt_row(zzinv[:, 0:6], zzrow[:, 0:6], 6, iscr, fscr)
            # fold both 1/sqrt(Z) factors into the 96-value At row below
            nc.vector.tensor_mul(zzinv[:, 6:8], zzinv[:, 0:2], zzinv[:, 3:5])
            nc.vector.tensor_mul(zzinv[:, 2:3], zzinv[:, 2:3], zzinv[:, 5:6])
            AsA = cpool.tile([128, 3, 128], F32)
            nc.vector.tensor_mul(AsA[:], ehraw[:], vsaff[:])
            vtaff = cpool.tile([1, 96], F32)
            nc.vector.tensor_mul(
                vtaff[:].rearrange("p (c t) -> p c t", c=3),
                ktrow[:, 96:192].rearrange("p (c t) -> p c t", c=3),
                arow[:, 0:3].unsqueeze(2).broadcast_to([1, 3, 32]))
            nc.vector.tensor_add(
                vtaff[:].rearrange("p (c t) -> p c t", c=3),
                vtaff[:].rearrange("p (c t) -> p c t", c=3),
                arow[:, 9:12].unsqueeze(2).broadcast_to([1, 3, 32]))
            atrow = cpool.tile([1, 96], F32)
            nc.vector.tensor_mul(atrow[:], ehalft[:], vtaff[:])
            # zz2 = [Z0*Zt0, Z1*Zt1, Z2*Zt2]^-1/2 lives at zzinv[6],[7],[2]
            nc.vector.tensor_mul(
                atrow[:, 0:32],
                atrow[:, 0:32],
                zzinv[:, 6:7].broadcast_to([1, 32]))
            nc.vector.tensor_mul(
                atrow[:, 32:64],
                atrow[:, 32:64],
                zzinv[:, 7:8].broadcast_to([1, 32]))
            nc.vector.tensor_mul(
                atrow[:, 64:96],
                atrow[:, 64:96],
                zzinv[:, 2:3].broadcast_to([1, 32]))
            atrep = cpool.tile([128, 96], F32)
            ps_at = pp.tile([128, 96], F32, tag="mm")
            nc.tensor.matmul(ps_at[:], lhsT=ones_row[:], rhs=atrow[:],
                             start=True, stop=True)
            nc.vector.tensor_copy(atrep[:], ps_at[:])

            # ---- outer product (bf16, DVE only) + contiguous output DMA ----
            # uneven chunks: a small final chunk keeps the tail DMA short
            for t0, tn in ((0, 10), (10, 10), (20, 10), (30, 2)):
                ost = wpool.tile([128, 3, tn, 128], BF16, tag=f"ost{tn}")
                nc.vector.tensor_tensor(
                    ost[:],
                    AsA[:].unsqueeze(2).broadcast_to([128, 3, tn, 128]),
                    atrep[:].rearrange("p (c t) -> p c t", c=3).unsqueeze(3)
                         [:, :, t0:t0 + tn, :].broadcast_to([128, 3, tn, 128]),
                    ALU.mult)
                nc.sync.dma_start(out_d[:, :, t0:t0 + tn, :], ost[:])

    nc.compile()
    return nc


def _in_maps(inputs, consts):
    x = np.asarray(inputs['x'], np.float32)
    maps = []
    for b in range(N_CORES):
        xp = np.zeros((128, 32, 130), np.float32)
        xp[:, :, 1:129] = x[b, 0].transpose(1, 0, 2)
        maps.append(dict(
            xin=xp, bandw=consts['bandw'], bandt=consts['bandt'],
            kvs_lhst=consts['kvs_lhst'], qsum=consts['qsum'],
            wkvt4=consts['wkvt4'], crow=consts['crow'],
        ))
    return maps


def _postprocess(res):
    outs = []
    for b in range(N_CORES):
        o = np.asarray(res.results[b]['out'])          # [h, c, t, w] bf16
        outs.append(o.astype(np.float32).transpose(1, 2, 0, 3))
    return np.stack(outs, axis=0)                      # [b, c, t, h, w]


def kernel(**inputs) -> np.ndarray:
    from concourse.bass_utils import run_bass_kernel_spmd
    consts = _host_constants(inputs)
    nc = build_program(consts['scal'])
    maps = _in_maps(inputs, consts)
    res = run_bass_kernel_spmd(nc, maps, list(range(N_CORES)))
    return _postprocess(res)
